# revision 1
# baseline (speedup 1.0000x reference)
"""DGCNN encoder (4x GraphConv + SortPooling) as a dense-adjacency Bass kernel.

Sharding: graph-level data parallelism. 8 cores x 4 graphs each.
Host prep: edge lists -> per-graph dense normalized adjacency (bf16),
features transposed. All feature math runs on device; per-core [4,1024]
outputs are concatenated on host (outputs are disjoint, no collective).
"""

import os
import sys

import numpy as np

sys.path.insert(0, "/opt/trn_rl_repo")

import ml_dtypes  # noqa: E402

from concourse import bass, bacc, mybir, tile  # noqa: E402
from concourse import bass_utils  # noqa: E402

# Walrus elides redundant PE weight loads only with --enable-ldw-opt=true;
# concourse pins it false. Flip it for this kernel's compile (A/B via env).
if os.environ.get("KERNEL_LDW_OPT", "0") == "1" and not getattr(
    bass_utils, "_ldw_patched", False
):
    _orig_run_command = bass_utils.run_command

    def _run_command_ldw(cmd, *a, **kw):
        if isinstance(cmd, list):
            cmd = [
                "--enable-ldw-opt=true" if c == "--enable-ldw-opt=false" else c
                for c in cmd
            ]
        return _orig_run_command(cmd, *a, **kw)

    bass_utils.run_command = _run_command_ldw
    bass_utils._ldw_patched = True

# Problem constants (hardcoded per spec; kernel.py must be self-contained).
B, N, DEG = 32, 2048, 16
IN_DIM = 128
HID = [128, 128, 128, 64]
K = 16
NCORES = 8
GPC = B // NCORES           # graphs per core = 4
NODES = GPC * N             # nodes per core = 8192
NCH = N // 128              # node chunks per graph = 16

F32 = mybir.dt.float32
BF16 = mybir.dt.bfloat16
I16 = mybir.dt.int16
I32 = mybir.dt.int32
U32 = mybir.dt.uint32

LAST = {"exec_time_ns": None}
_CACHE = {}


def _build_graph():
    """Build the per-core SPMD Bass graph (identical on all cores)."""
    nc = bacc.Bacc(
        "TRN2",
        target_bir_lowering=False,
        debug=False,
        enable_asserts=False,
        num_devices=NCORES,
    )

    featT = nc.dram_tensor("featT", [128, NODES], BF16, kind="ExternalInput")
    at_in = nc.dram_tensor("at", [GPC, 128, NCH, N], BF16, kind="ExternalInput")
    w_in = [
        nc.dram_tensor(f"w{i+1}", [128, HID[i]], BF16, kind="ExternalInput")
        for i in range(4)
    ]
    b_in = [
        nc.dram_tensor(f"b{i+1}", [HID[i], 1], F32, kind="ExternalInput")
        for i in range(4)
    ]
    ident_in = nc.dram_tensor("ident", [128, 128], F32, kind="ExternalInput")
    rep_in = nc.dram_tensor("repmat", [16, 128], F32, kind="ExternalInput")
    out_dram = nc.dram_tensor("out", [GPC, K * 64], F32, kind="ExternalOutput")
    h4_hbm = nc.dram_tensor("h4hbm", [NODES, 64], F32)

    relu = mybir.ActivationFunctionType.Relu
    mxo = mybir.AluOpType.max

    with tile.TileContext(nc) as tc:
        with (
            tc.tile_pool(name="const", bufs=1) as constp,
            tc.tile_pool(name="atp", bufs=2) as atp,
            tc.tile_pool(name="htp", bufs=2) as htp,
            tc.tile_pool(name="hnp", bufs=8) as hnp,
            tc.tile_pool(name="h4np", bufs=1) as h4np,
            tc.tile_pool(name="sortp", bufs=1) as sortp,
            tc.tile_pool(name="h4tp", bufs=1) as h4tp,
            tc.tile_pool(name="aggps", bufs=1, space="PSUM") as aggps,
            tc.tile_pool(name="wps", bufs=2, space="PSUM") as wps,
            tc.tile_pool(name="tps", bufs=2, space="PSUM") as tps,
        ):
            # ---- loads ordered so graph-0 compute starts ASAP ----
            # critical path (first W matmuls) on the fast HWDGE queue, first
            ft = constp.tile([128, NODES], BF16, tag="featT")
            nc.sync.dma_start(out=ft[:, 0:N], in_=featT[:, 0:N])
            wt = []
            for i in range(4):
                w = constp.tile([128, HID[i]], BF16, tag=f"w{i}")
                nc.sync.dma_start(out=w[:, :], in_=w_in[i][:, :])
                wt.append(w)

            at_tiles = [None] * GPC
            at0 = atp.tile([128, NCH, N], BF16, tag="at")
            for q in range(4):
                nc.sync.dma_start(
                    out=at0[:, q * 4 : (q + 1) * 4, :],
                    in_=at_in[0, :, q * 4 : (q + 1) * 4, :],
                )
            at_tiles[0] = at0

            bt = []
            for i in range(4):
                bb = constp.tile([HID[i], 1], F32, tag=f"b{i}")
                nc.gpsimd.dma_start(out=bb[:, :], in_=b_in[i][:, :])
                bt.append(bb)
            for g in range(1, GPC):
                nc.gpsimd.dma_start(
                    out=ft[:, g * N : (g + 1) * N], in_=featT[:, g * N : (g + 1) * N]
                )
            ident = constp.tile([128, 128], F32, tag="ident")
            nc.gpsimd.dma_start(out=ident[:, :], in_=ident_in[:, :])
            repm = constp.tile([16, 128], F32, tag="repm")
            nc.gpsimd.dma_start(out=repm[:, :], in_=rep_in[:, :])

            # sortpool state (graph g's node-max row lives on partition 32g);
            # init on the (idle) vector engine, off the startup critical path
            mvall = constp.tile([128, N], F32, tag="mvall")
            nc.vector.memset(mvall[:, :], -1e30)
            offsp = sortp.tile([128, 1], I32, tag="offsp")
            nc.gpsimd.iota(offsp[:, :], pattern=[[0, 1]], base=0, channel_multiplier=64)
            offspf = sortp.tile([128, 1], F32, tag="offspf")
            nc.vector.tensor_copy(offspf[:, :], offsp[:, :])
            idx16 = sortp.tile([128, 8], I16, tag="idx16")
            nc.vector.memset(idx16[:, :], -1)

            for g in range(GPC):
                if g > 0:
                    at = atp.tile([128, NCH, N], BF16, tag="at")
                    for q in range(4):
                        nc.sync.dma_start(
                            out=at[:, q * 4 : (q + 1) * 4, :],
                            in_=at_in[g, :, q * 4 : (q + 1) * 4, :],
                        )
                else:
                    at = at_tiles[0]

                hT_prev = None  # layer input, transposed [Din<=128, N] bf16
                for li in range(4):
                    dout = HID[li]
                    # ---- W matmul: h'n[c] = (hT chunk).T @ W -> node-major ----
                    hn_tiles = []
                    for cq in range(NCH // 4):
                        wp = wps.tile([128, 4, dout], F32, tag="wp")
                        for i in range(4):
                            c = cq * 4 + i
                            if li == 0:
                                lhsT = ft[:, g * N + c * 128 : g * N + (c + 1) * 128]
                            else:
                                lhsT = hT_prev[:, c * 128 : (c + 1) * 128]
                            nc.tensor.matmul(
                                wp[:, i, :], lhsT, wt[li][:, :dout],
                                start=True, stop=True,
                            )
                        hn = hnp.tile([128, 4, 128], BF16, tag="hn")
                        nc.vector.tensor_copy(hn[:, :, :dout], wp[:, :, :])
                        hn_tiles.append(hn)

                    # ---- aggregation: aggT[d, dst] += h'n[src] @ AT ----
                    agg = aggps.tile([128, N], F32, tag="agg")
                    for c in range(NCH):
                        lhsT = hn_tiles[c // 4][:, c % 4, :dout]
                        for dsp in range(N // 512):
                            nc.tensor.matmul(
                                agg[:dout, dsp * 512 : (dsp + 1) * 512],
                                lhsT,
                                at[:, c, dsp * 512 : (dsp + 1) * 512],
                                start=(c == 0),
                                stop=(c == NCH - 1),
                            )

                    # ---- bias + relu, split 4x so next layer starts early ----
                    if li < 3:
                        hT = htp.tile([128, N], BF16, tag="ht")
                        dst_t = hT
                    else:
                        dst_t = h4tp.tile([64, N], F32, tag="h4t")
                    for q in range(4):
                        sl = slice(q * 512, (q + 1) * 512)
                        nc.scalar.activation(
                            dst_t[:dout, sl], agg[:dout, sl], relu,
                            bias=bt[li][:, :],
                        )
                    if li < 3:
                        hT_prev = hT
                    else:
                        h4T = dst_t

                # ---- layer-4 post: transpose to node-major, rowmax, HBM ----
                h4n = h4np.tile([128, NCH, 64], F32, tag="h4n")
                for c in range(NCH):
                    tp = tps.tile([128, 128], F32, tag="tp")
                    nc.tensor.transpose(
                        tp[:, :64], h4T[:, c * 128 : (c + 1) * 128], ident[:64, :64]
                    )
                    nc.vector.tensor_copy(h4n[:, c, :], tp[:, :64])
                mcg = sortp.tile([128, NCH], F32, tag="mcg")
                nc.vector.tensor_reduce(
                    mcg[:, :], h4n[:, :, :], axis=mybir.AxisListType.X, op=mxo
                )
                nc.sync.dma_start(
                    out=h4_hbm[g * N : (g + 1) * N, :].rearrange(
                        "(c p) f -> p c f", p=128
                    ),
                    in_=h4n[:, :, :],
                )
                # node-max row -> mvall partition 32g (hidden under compute)
                tpg = tps.tile([128, 128], F32, tag="tp")
                nc.tensor.transpose(tpg[:NCH, :], mcg[:, :], ident[:, :])
                mtg = sortp.tile([NCH, 128], F32, tag="mtg")
                nc.vector.tensor_copy(mtg[:, :], tpg[:NCH, :])
                p0 = 32 * g
                nc.sync.dma_start(
                    out=mvall[p0 : p0 + 1, :].rearrange("o (c j) -> o c j", j=128),
                    in_=mtg[:, :],
                )

            # ================= batched top-16 + gather + sort tail =========
            mxs = sortp.tile([128, 8], F32, tag="mxs")
            mis = sortp.tile([128, 16], U32, tag="mis")
            nc.vector.max_with_indices(mxs[:, :], mis[:, 0:8], mvall[:, :])
            mv2 = sortp.tile([128, N], F32, tag="mv2")
            nc.vector.match_replace(mv2[:, :], mxs[:, :], mvall[:, :], -1e30)
            nc.vector.max_with_indices(mxs[:, :], mis[:, 8:16], mv2[:, :])

            idxf32 = sortp.tile([128, 16], F32, tag="idxf32")
            nc.vector.tensor_copy(idxf32[:, :], mis[:, :])
            idxo = sortp.tile([128, 16], F32, tag="idxo")
            nc.vector.tensor_scalar(
                idxo[:, :], idxf32[:, :], offspf[:, :], None, op0=mybir.AluOpType.add
            )
            tpi = tps.tile([128, 128], F32, tag="tp")
            nc.tensor.transpose(tpi[:16, :], idxo[:, :], ident[:, :])
            t1s = sortp.tile([16, 128], F32, tag="t1s")
            nc.vector.tensor_copy(t1s[:, :], tpi[:16, :])
            # columns {0,32,64,96} hold graphs 0..3; broadcast to 128 rows via PE
            sel = t1s[:, :].rearrange("r (a b) -> r a b", b=32)[:, :, 0]
            tpr = tps.tile([128, 128], F32, tag="tp")
            nc.tensor.matmul(tpr[:, :GPC], repm[:, :], sel, start=True, stop=True)
            nc.vector.tensor_copy(idx16[:, 0:GPC], tpr[:, :GPC])

            # gather the 64 selected node rows from HBM
            gath = sortp.tile([128, 1, 64], F32, tag="gath")
            nc.gpsimd.dma_gather(
                gath[:, :, :],
                h4_hbm[:, :],
                idx16[:, :],
                num_idxs=128,
                num_idxs_reg=64,
                elem_size=64,
            )

            # ---- ascending sort of 64 values per row via max8 rounds on -x ----
            neg = sortp.tile([64, 64], F32, tag="neg")
            nc.vector.tensor_scalar(
                neg[:, :], gath[:64, 0, :], -1.0, None, op0=mybir.AluOpType.mult
            )
            desc = sortp.tile([64, 64], F32, tag="desc")
            pp0 = sortp.tile([64, 64], F32, tag="pp0")
            pp1 = sortp.tile([64, 64], F32, tag="pp1")
            pp = [pp0, pp1]
            cur = neg
            for r in range(8):
                nc.vector.max(desc[:, r * 8 : (r + 1) * 8], cur[:, :])
                if r < 7:
                    nxt = pp[r % 2]
                    nc.vector.match_replace(
                        nxt[:, :], desc[:, r * 8 : (r + 1) * 8], cur[:, :], -1e30
                    )
                    cur = nxt
            asc = sortp.tile([64, 64], F32, tag="asc")
            nc.vector.tensor_scalar(
                asc[:, :], desc[:, :], -1.0, None, op0=mybir.AluOpType.mult
            )

            # ---- write output [4, 1024] ----
            nc.sync.dma_start(
                out=out_dram[:, :].rearrange("g (r f) -> (g r) f", f=64),
                in_=asc[:, :],
            )

    nc.compile()
    return nc


def _host_prep(inputs):
    """Shard + structural preprocessing: per-graph normalized dense adjacency."""
    feats = np.asarray(inputs["features"], np.float32)
    src = np.asarray(inputs["src"], np.int64)
    dst = np.asarray(inputs["dst"], np.int64)
    n_rand = B * N * DEG
    rs, rd = src[:n_rand], dst[:n_rand]

    ident = np.eye(128, dtype=np.float32)
    repmat = np.tile(np.eye(16, dtype=np.float32), (1, 8))  # [16, 128]
    in_maps = []
    for core in range(NCORES):
        at_core = np.empty((GPC, 128, NCH, N), dtype=ml_dtypes.bfloat16)
        for g in range(GPC):
            gb = core * GPC + g
            s = rs[gb * N * DEG : (gb + 1) * N * DEG] - gb * N
            d = rd[gb * N * DEG : (gb + 1) * N * DEG] - gb * N
            cnt = np.bincount(s * N + d, minlength=N * N).astype(np.float32)
            cnt = cnt.reshape(N, N)
            np.fill_diagonal(cnt, np.diagonal(cnt) + 1.0)  # self loops
            odeg = cnt.sum(axis=1)
            ideg = cnt.sum(axis=0)
            od = (1.0 / np.sqrt(np.maximum(odeg, 1.0))).astype(np.float32)
            idg = (1.0 / np.sqrt(np.maximum(ideg, 1.0))).astype(np.float32)
            a = (od[:, None] * cnt) * idg[None, :]
            # [src, dst] -> [128, 16, 2048]: at[p, c, :] = a[c*128+p, :]
            at_core[g] = (
                a.reshape(NCH, 128, N).transpose(1, 0, 2).astype(ml_dtypes.bfloat16)
            )
        fshard = np.ascontiguousarray(
            feats[core * NODES : (core + 1) * NODES].T
        ).astype(ml_dtypes.bfloat16)
        m = {"featT": fshard, "at": at_core, "ident": ident, "repmat": repmat}
        for i in range(4):
            m[f"w{i+1}"] = np.asarray(inputs[f"W{i+1}"], np.float32).astype(
                ml_dtypes.bfloat16
            )
            m[f"b{i+1}"] = np.asarray(inputs[f"b{i+1}"], np.float32).reshape(-1, 1)
        in_maps.append(m)
    return in_maps


def kernel(**inputs):
    if "nc" not in _CACHE:
        _CACHE["nc"] = _build_graph()
    nc = _CACHE["nc"]
    in_maps = _host_prep(inputs)
    trace = bool(int(os.environ.get("KERNEL_TRACE", "0")))
    res = bass_utils.run_bass_kernel_spmd(
        nc, in_maps, core_ids=list(range(NCORES)), trace=trace
    )
    LAST["exec_time_ns"] = res.exec_time_ns
    out = np.concatenate([res.results[i]["out"] for i in range(NCORES)], axis=0)
    return out.astype(np.float32)



# revision 15
# speedup vs baseline: 1.3874x; 1.3874x over previous
"""DGCNN encoder (4x GraphConv + SortPooling) — fp8 DoubleRow Bass kernel.

Sharding: graph-level data parallelism, 8 cores x 4 graphs each.

Math per graph (DGL GraphConv norm='both', biases zero — checked on host):
  h_{l+1} = idg ∘ relu( C^T fp8(oi ∘ (h_l W_l)) + b )
with C the integer edge-count matrix (exact in fp8-e4m3), oi = od*idg
per-node scales applied in f32 during the PSUM->SBUF cast (idg of layer l
deferred into layer l+1's cast; valid for b=0, host falls back to a dense
bf16 graph otherwise).  The O(N^2 D) aggregation runs as fp8 DoubleRow
matmuls: two 128-src chunks contracted per pass at 2x bf16 rate.

SortPooling: the fp8 h4 only RANKS nodes (GPSIMD partition-max + idg row
scale + max8 rounds -> top-32 per graph).  Candidates are re-aggregated
exactly in f32 from bf16 integer count columns fetched with a transposed
dma_gather (idg value smuggled in each row's chunk 16), re-ranked, and the
top-16 rows are gathered, row-sorted ascending and written out.
"""

import os
import sys

import numpy as np

sys.path.insert(0, "/opt/trn_rl_repo")

import ml_dtypes  # noqa: E402

from concourse import bass, bacc, mybir, tile  # noqa: E402
from concourse import bass_isa  # noqa: E402
from concourse import bass_utils  # noqa: E402

B, N, DEG = 32, 2048, 16
IN_DIM = 128
HID = [128, 128, 128, 64]
K = 16
NCORES = 8
GPC = B // NCORES           # graphs per core = 4
NODES = GPC * N             # nodes per core = 8192
NCH = N // 128              # node chunks per graph = 16
NCAND = 64                  # sortpool recompute candidates per graph
NSEG = 4                    # key-vector segments per graph (parallel top-16)
SEGW = N // NSEG            # 512 keys per segment
ROWE = N + 128              # cntTs row: 2048 counts | idg | 127 pad
AGG_ONE_MM = False          # bank-crossing matmul fails the ISA check

F32 = mybir.dt.float32
BF16 = mybir.dt.bfloat16
FP8 = mybir.dt.float8e4
I16 = mybir.dt.int16
U32 = mybir.dt.uint32
DR = mybir.MatmulPerfMode.DoubleRow

LAST = {"exec_time_ns": None}
_CACHE = {}


def _build_graph_fast():
    nc = bacc.Bacc(
        "TRN2",
        target_bir_lowering=False,
        debug=False,
        enable_asserts=False,
        num_devices=NCORES,
    )

    featT = nc.dram_tensor("featT", [128, NODES], BF16, kind="ExternalInput")
    at_in = nc.dram_tensor("at", [GPC, 128, NCH, N], FP8, kind="ExternalInput")
    cntTs = nc.dram_tensor("cntTs", [NODES, ROWE], BF16, kind="ExternalInput")
    oiT_in = nc.dram_tensor("oiT", [128, GPC * NCH], F32, kind="ExternalInput")
    idgR_in = nc.dram_tensor("idgR", [1, NODES], BF16, kind="ExternalInput")
    w_in = [
        nc.dram_tensor(f"w{i+1}", [128, HID[i]], BF16, kind="ExternalInput")
        for i in range(4)
    ]
    b_in = [
        nc.dram_tensor(f"b{i+1}", [HID[i], 1], F32, kind="ExternalInput")
        for i in range(4)
    ]
    ident_in = nc.dram_tensor("ident", [128, 128], F32, kind="ExternalInput")
    rep_in = nc.dram_tensor("repmat", [16, 128], F32, kind="ExternalInput")
    out_dram = nc.dram_tensor("out", [GPC, K * 64], F32, kind="ExternalOutput")
    hex_hbm = nc.dram_tensor("hexhbm", [GPC * NCAND, 64], F32)

    relu = mybir.ActivationFunctionType.Relu
    mxo = mybir.AluOpType.max
    mlt = mybir.AluOpType.mult
    add = mybir.AluOpType.add

    with tile.TileContext(nc) as tc:
        with (
            tc.tile_pool(name="const", bufs=1) as constp,
            tc.tile_pool(name="atp", bufs=2) as atp,
            tc.tile_pool(name="htp", bufs=2) as htp,
            tc.tile_pool(name="hnp", bufs=2) as hnp,
            tc.tile_pool(name="hodp", bufs=2) as hodp,
            tc.tile_pool(name="h4tp", bufs=1) as h4tp,
            tc.tile_pool(name="tailp", bufs=1) as tailp,
            tc.tile_pool(name="mvgp", bufs=2) as mvgp,
            tc.tile_pool(name="aggps", bufs=1, space="PSUM") as aggps,
            tc.tile_pool(name="wps", bufs=2, space="PSUM") as wps,
            tc.tile_pool(name="tps", bufs=1, space="PSUM") as tps,
            tc.tile_pool(name="rexps", bufs=1, space="PSUM") as rexps,
        ):
            # ---- loads: critical path first on the fast queue ----
            ft = constp.tile([128, NODES], BF16, tag="featT")
            nc.sync.dma_start(out=ft[:, 0:1024], in_=featT[:, 0:1024])
            wt = []
            for i in range(4):
                w = constp.tile([128, HID[i]], BF16, tag=f"w{i}")
                nc.sync.dma_start(out=w[:, :], in_=w_in[i][:, :])
                wt.append(w)

            at_tiles = [None] * GPC
            at0 = atp.tile([128, NCH, N], FP8, tag="at")
            nc.sync.dma_start(out=at0[:, 0:4, :], in_=at_in[0, :, 0:4, :])
            nc.sync.dma_start(out=ft[:, 1024:N], in_=featT[:, 1024:N])
            for q in range(1, 4):
                nc.sync.dma_start(
                    out=at0[:, q * 4 : (q + 1) * 4, :],
                    in_=at_in[0, :, q * 4 : (q + 1) * 4, :],
                )
            at_tiles[0] = at0

            bt = []
            for i in range(4):
                bb = constp.tile([HID[i], 1], F32, tag=f"b{i}")
                nc.gpsimd.dma_start(out=bb[:, :], in_=b_in[i][:, :])
                bt.append(bb)
            for g in range(1, GPC):
                nc.gpsimd.dma_start(
                    out=ft[:, g * N : (g + 1) * N], in_=featT[:, g * N : (g + 1) * N]
                )
            oiT = constp.tile([128, GPC * NCH], F32, tag="oiT")
            nc.gpsimd.dma_start(out=oiT[:, :], in_=oiT_in[:, :])
            idgR = constp.tile([1, NODES], BF16, tag="idgR")
            nc.gpsimd.dma_start(out=idgR[:, :], in_=idgR_in[:, :])
            ident = constp.tile([128, 128], F32, tag="ident")
            nc.gpsimd.dma_start(out=ident[:, :], in_=ident_in[:, :])
            repm = constp.tile([16, 128], F32, tag="repm")
            nc.gpsimd.dma_start(out=repm[:, :], in_=rep_in[:, :])

            soff = tailp.tile([128, 1], mybir.dt.int32, tag="soff")
            nc.gpsimd.iota(
                soff[:, :], pattern=[[0, 1]], base=0, channel_multiplier=SEGW
            )
            sofff = tailp.tile([128, 1], F32, tag="sofff")
            nc.vector.tensor_copy(sofff[:, :], soff[:, :])

            st = {}

            def phase_a(g):
                """rank keys: partition-max of h4T (GPSIMD), idg row scale,
                then split into NSEG partition rows for parallel top-16."""
                h4T = st[g, "h4T"]
                pmax = tailp.tile([64, N], F32, tag="pmax")
                nc.gpsimd.partition_all_reduce(
                    pmax[:, :], h4T[:, :], channels=64,
                    reduce_op=bass_isa.ReduceOp.max,
                )
                mvg = mvgp.tile([1, N], F32, tag="mvg")
                nc.vector.tensor_tensor(
                    mvg[0:1, :], pmax[0:1, :],
                    idgR[0:1, g * N : (g + 1) * N], mlt,
                )
                mvseg = mvgp.tile([NSEG, SEGW], F32, tag="mvseg")
                nc.sync.dma_start(
                    out=mvseg[:, :],
                    in_=mvg[0:1, :].rearrange("o (s j) -> o s j", j=SEGW),
                )
                st[g, "mvseg"] = mvseg

            def phase_b1(g):
                """per-segment top-16 (guaranteed superset of graph top-16)."""
                mvseg = st[g, "mvseg"]
                mx4 = tailp.tile([NSEG, 8], F32, tag="mx4")
                mi4 = tailp.tile([NSEG, 16], U32, tag=f"mi4{g}")
                mva = tailp.tile([NSEG, SEGW], F32, tag="mva")
                nc.vector.max_with_indices(mx4[:, :], mi4[:, 0:8], mvseg[:, :])
                nc.vector.match_replace(mva[:, :], mx4[:, :], mvseg[:, :], -1e30)
                nc.vector.max_with_indices(mx4[:, :], mi4[:, 8:16], mva[:, :])
                idxf = tailp.tile([NSEG, 16], F32, tag=f"idxf{g}")
                nc.vector.tensor_scalar(
                    idxf[:, :], mi4[:, :], sofff[:NSEG, :], None, op0=add
                )
                st[g, "idxf"] = idxf

            def phase_b2(g):
                """16-wrapped idx tile + transposed gather of count columns."""
                idxf = st[g, "idxf"]
                tp = tps.tile([128, 128], F32, tag="tp")
                nc.tensor.transpose(
                    tp[:16, 0:NSEG], idxf[0:NSEG, 0:16], ident[:NSEG, :NSEG]
                )
                idx2 = tailp.tile([16, NSEG], F32, tag="idx2")
                nc.vector.tensor_copy(idx2[:, :], tp[:16, 0:NSEG])
                tpr = tps.tile([128, 128], F32, tag="tp")
                nc.tensor.matmul(
                    tpr[:, 0:NSEG], repm[:, :], idx2[:, :], start=True, stop=True
                )
                idxg = tailp.tile([128, 8], I16, tag="idxg")
                nc.vector.memset(idxg[:, :], 0)
                nc.vector.tensor_copy(idxg[:, 0:NSEG], tpr[:, 0:NSEG])
                gathT = tailp.tile([128, NCH + 1, 128], BF16, tag="gathT")
                nc.gpsimd.dma_gather(
                    gathT[:, :, :],
                    cntTs[g * N : (g + 1) * N, :],
                    idxg[:, :],
                    num_idxs=128,
                    num_idxs_reg=128,
                    elem_size=ROWE,
                    transpose=True,
                )
                st[g, "gathT"] = gathT

            def phase_c(g):
                """exact f32 re-aggregation of candidates + rank + store."""
                gathT = st[g, "gathT"]
                hod4 = st[g, "hod4"]
                cntc = tailp.tile([128, NCH, NCAND], F32, tag="cntc")
                nc.vector.tensor_copy(cntc[:, :, :], gathT[:, 0:NCH, 0:NCAND])
                idgcf = tailp.tile([1, NCAND], F32, tag="idgcf")
                nc.vector.tensor_copy(idgcf[:, :], gathT[0:1, NCH, 0:NCAND])
                tpi = tps.tile([128, 128], F32, tag="tp")
                nc.tensor.transpose(
                    tpi[:NCAND, 0:1], idgcf[0:1, :], ident[:1, :1]
                )
                idgc32 = tailp.tile([NCAND, 1], F32, tag="idgc32")
                nc.vector.tensor_copy(idgc32[:, :], tpi[:NCAND, 0:1])
                rex = rexps.tile([NCAND, 64], F32, tag="rex")
                for c in range(NCH):
                    nc.tensor.matmul(
                        rex[:, :],
                        cntc[:, c, :],
                        hod4[:, c, :],
                        start=(c == 0),
                        stop=(c == NCH - 1),
                    )
                hex_t = tailp.tile([NCAND, 64], F32, tag="hex")
                nc.scalar.activation(
                    hex_t[:, :], rex[:, :], relu, scale=idgc32[:, :]
                )
                nc.sync.dma_start(
                    out=hex_hbm[g * NCAND : (g + 1) * NCAND, :], in_=hex_t[:, :]
                )
                hkey = tailp.tile([NCAND, 1], F32, tag="hkey")
                nc.vector.tensor_reduce(
                    hkey[:, :], hex_t[:, :], axis=mybir.AxisListType.X, op=mxo
                )
                tpk = tps.tile([128, 128], F32, tag="tp")
                nc.tensor.transpose(
                    tpk[0:1, 0:NCAND], hkey[:, 0:1], ident[:NCAND, :NCAND]
                )
                hkr = tailp.tile([1, NCAND], F32, tag="hkr")
                nc.vector.tensor_copy(hkr[:, :], tpk[0:1, 0:NCAND])
                kxs = tailp.tile([1, 8], F32, tag="kxs")
                kis = tailp.tile([1, K], U32, tag="kis")
                kva = tailp.tile([1, NCAND], F32, tag="kva")
                cur = hkr
                for r in range(K // 8):
                    nc.vector.max_with_indices(
                        kxs[:, :], kis[:, 8 * r : 8 * r + 8], cur[:, :]
                    )
                    if r < K // 8 - 1:
                        nc.vector.match_replace(kva[:, :], kxs[:, :], cur[:, :], -1e30)
                        cur = kva
                # ---- per-graph finalize: gather the 16 rows, sort, write ----
                kisf = tailp.tile([1, K], F32, tag="kisf")
                nc.vector.tensor_copy(kisf[:, :], kis[:, :])
                tpq = tps.tile([128, 128], F32, tag="tp")
                nc.tensor.transpose(tpq[:K, 0:1], kisf[0:1, :], ident[:1, :1])
                kcol = tailp.tile([K, 1], F32, tag="kcol")
                nc.vector.tensor_copy(kcol[:, :], tpq[:K, 0:1])
                tpr = tps.tile([128, 128], F32, tag="tp")
                nc.tensor.matmul(
                    tpr[:, 0:1], repm[:, :], kcol[:, 0:1], start=True, stop=True
                )
                idxq = tailp.tile([128, 1], I16, tag="idxq")
                nc.vector.tensor_copy(idxq[:, :], tpr[:, 0:1])
                gath16 = tailp.tile([128, 1, 64], F32, tag="gath16")
                nc.gpsimd.dma_gather(
                    gath16[:, :, :],
                    hex_hbm[g * NCAND : (g + 1) * NCAND, :],
                    idxq[:, :],
                    num_idxs=16,
                    num_idxs_reg=16,
                    elem_size=64,
                )
                neg = tailp.tile([K, 64], F32, tag="neg")
                nc.vector.tensor_scalar(
                    neg[:, :], gath16[:K, 0, :], -1.0, None, op0=mlt
                )
                desc = tailp.tile([K, 64], F32, tag="desc")
                pp0 = tailp.tile([K, 64], F32, tag="pp0")
                pp1 = tailp.tile([K, 64], F32, tag="pp1")
                pp = [pp0, pp1]
                cur = neg
                for r in range(8):
                    nc.vector.max(desc[:, r * 8 : (r + 1) * 8], cur[:, :])
                    if r < 7:
                        nxt = pp[r % 2]
                        nc.vector.match_replace(
                            nxt[:, :], desc[:, r * 8 : (r + 1) * 8], cur[:, :], -1e30
                        )
                        cur = nxt
                asc = tailp.tile([K, 64], F32, tag="asc")
                nc.vector.tensor_scalar(asc[:, :], desc[:, :], -1.0, None, op0=mlt)
                nc.sync.dma_start(
                    out=out_dram[g : g + 1, :].rearrange("o (r f) -> (o r) f", f=64),
                    in_=asc[:, :],
                )

            # ================= main per-graph pipeline =================
            for g in range(GPC):
                if g > 0:
                    at = atp.tile([128, NCH, N], FP8, tag="at")
                    for q in range(4):
                        nc.sync.dma_start(
                            out=at[:, q * 4 : (q + 1) * 4, :],
                            in_=at_in[g, :, q * 4 : (q + 1) * 4, :],
                        )
                else:
                    at = at_tiles[0]

                hT_prev = None
                for li in range(4):
                    dout = HID[li]
                    hn = hnp.tile([128, NCH, 128], FP8, tag="hn")
                    if li == 3:
                        hod4 = hodp.tile([128, NCH, 64], F32, tag="hod4")
                        st[g, "hod4"] = hod4
                    for cq in range(NCH // 4):
                        wp = wps.tile([128, 4, dout], F32, tag="wp")
                        for i in range(4):
                            c = cq * 4 + i
                            if li == 0:
                                lhsT = ft[:, g * N + c * 128 : g * N + (c + 1) * 128]
                            else:
                                lhsT = hT_prev[:, c * 128 : (c + 1) * 128]
                            nc.tensor.matmul(
                                wp[:, i, :], lhsT, wt[li][:, :dout],
                                start=True, stop=True,
                            )
                        sc = oiT[:, g * NCH + cq * 4 : g * NCH + cq * 4 + 4]
                        scb = sc[:, :, None].to_broadcast([128, 4, dout])
                        if li == 0:
                            nc.vector.tensor_copy(
                                hn[:, cq * 4 : (cq + 1) * 4, :dout], wp[:, :, :]
                            )
                        else:
                            nc.vector.tensor_tensor(
                                hn[:, cq * 4 : (cq + 1) * 4, :dout],
                                wp[:, :, :], scb, mlt,
                            )
                        if li == 3:
                            nc.vector.tensor_tensor(
                                hod4[:, cq * 4 : (cq + 1) * 4, :],
                                wp[:, :, :], scb, mlt,
                            )

                    # ---- fp8 DoubleRow aggregation, 8 chunk-pairs ----
                    agg = aggps.tile([128, N], F32, tag="agg")
                    for pr in range(NCH // 2):
                        if AGG_ONE_MM:
                            nc.tensor.matmul(
                                agg[:dout, :],
                                hn[:, 2 * pr : 2 * pr + 2, :dout],
                                at[:, 2 * pr : 2 * pr + 2, :],
                                start=(pr == 0),
                                stop=(pr == NCH // 2 - 1),
                                perf_mode=DR,
                            )
                        else:
                            for dsp in range(N // 512):
                                nc.tensor.matmul(
                                    agg[:dout, dsp * 512 : (dsp + 1) * 512],
                                    hn[:, 2 * pr : 2 * pr + 2, :dout],
                                    at[:, 2 * pr : 2 * pr + 2,
                                       dsp * 512 : (dsp + 1) * 512],
                                    start=(pr == 0),
                                    stop=(pr == NCH // 2 - 1),
                                    perf_mode=DR,
                                )

                    if li < 3:
                        hT = htp.tile([128, N], BF16, tag="ht")
                        dst_t = hT
                    else:
                        dst_t = h4tp.tile([64, N], F32, tag="h4t")
                    for q in range(4):
                        sl = slice(q * 512, (q + 1) * 512)
                        nc.scalar.activation(
                            dst_t[:dout, sl], agg[:dout, sl], relu, bias=bt[li][:, :]
                        )
                    if li < 3:
                        hT_prev = hT
                    else:
                        st[g, "h4T"] = dst_t

                    if g > 0:
                        if li == 0:
                            phase_b1(g - 1)
                        elif li == 1:
                            phase_b2(g - 1)
                        elif li == 2:
                            phase_c(g - 1)

                phase_a(g)

            phase_b1(GPC - 1)
            phase_b2(GPC - 1)
            phase_c(GPC - 1)

    nc.compile()
    return nc


def _host_prep_fast(inputs):
    feats = np.asarray(inputs["features"], np.float32)
    src = np.asarray(inputs["src"], np.int64)
    dst = np.asarray(inputs["dst"], np.int64)
    n_rand = B * N * DEG
    rs, rd = src[:n_rand], dst[:n_rand]

    ident = np.eye(128, dtype=np.float32)
    repmat = np.tile(np.eye(16, dtype=np.float32), (1, 8))
    in_maps = []
    max_cnt = 0.0
    for core in range(NCORES):
        at_core = np.empty((GPC, 128, NCH, N), dtype=ml_dtypes.float8_e4m3)
        cntTs = np.zeros((NODES, ROWE), dtype=ml_dtypes.bfloat16)
        oiT = np.empty((128, GPC * NCH), dtype=np.float32)
        idgR = np.empty((1, NODES), dtype=ml_dtypes.bfloat16)
        fodT = np.empty((128, NODES), dtype=ml_dtypes.bfloat16)
        for g in range(GPC):
            gb = core * GPC + g
            s = rs[gb * N * DEG : (gb + 1) * N * DEG] - gb * N
            d = rd[gb * N * DEG : (gb + 1) * N * DEG] - gb * N
            cnt = np.bincount(s * N + d, minlength=N * N).astype(np.float32)
            cnt = cnt.reshape(N, N)
            np.fill_diagonal(cnt, np.diagonal(cnt) + 1.0)
            max_cnt = max(max_cnt, float(cnt.max()))
            odeg = cnt.sum(axis=1)
            ideg = cnt.sum(axis=0)
            od = (1.0 / np.sqrt(np.maximum(odeg, 1.0))).astype(np.float32)
            idg = (1.0 / np.sqrt(np.maximum(ideg, 1.0))).astype(np.float32)
            at_core[g] = (
                cnt.reshape(NCH, 128, N).transpose(1, 0, 2)
                .astype(ml_dtypes.float8_e4m3)
            )
            cntTs[g * N : (g + 1) * N, 0:N] = cnt.T.astype(ml_dtypes.bfloat16)
            cntTs[g * N : (g + 1) * N, N] = idg.astype(ml_dtypes.bfloat16)
            oiT[:, g * NCH : (g + 1) * NCH] = (od * idg).reshape(NCH, 128).T
            idgR[0, g * N : (g + 1) * N] = idg.astype(ml_dtypes.bfloat16)
            fodT[:, g * N : (g + 1) * N] = (
                feats[gb * N : (gb + 1) * N].T * od[None, :]
            ).astype(ml_dtypes.bfloat16)
        m = {
            "featT": fodT, "at": at_core, "cntTs": cntTs,
            "oiT": oiT, "idgR": idgR, "ident": ident, "repmat": repmat,
        }
        for i in range(4):
            m[f"w{i+1}"] = np.asarray(inputs[f"W{i+1}"], np.float32).astype(
                ml_dtypes.bfloat16
            )
            m[f"b{i+1}"] = np.asarray(inputs[f"b{i+1}"], np.float32).reshape(-1, 1)
        in_maps.append(m)
    return in_maps, max_cnt


# ---------------- fallback: dense bf16 graph (arbitrary biases/counts) ------

def _build_graph_fallback():
    nc = bacc.Bacc(
        "TRN2",
        target_bir_lowering=False,
        debug=False,
        enable_asserts=False,
        num_devices=NCORES,
    )

    featT = nc.dram_tensor("featT", [128, NODES], BF16, kind="ExternalInput")
    at_in = nc.dram_tensor("at", [GPC, 128, NCH, N], BF16, kind="ExternalInput")
    w_in = [
        nc.dram_tensor(f"w{i+1}", [128, HID[i]], BF16, kind="ExternalInput")
        for i in range(4)
    ]
    b_in = [
        nc.dram_tensor(f"b{i+1}", [HID[i], 1], F32, kind="ExternalInput")
        for i in range(4)
    ]
    ident_in = nc.dram_tensor("ident", [128, 128], F32, kind="ExternalInput")
    rep_in = nc.dram_tensor("repmat", [16, 128], F32, kind="ExternalInput")
    out_dram = nc.dram_tensor("out", [GPC, K * 64], F32, kind="ExternalOutput")
    h4_hbm = nc.dram_tensor("h4hbm", [NODES, 64], F32)

    relu = mybir.ActivationFunctionType.Relu
    mxo = mybir.AluOpType.max
    I32 = mybir.dt.int32

    with tile.TileContext(nc) as tc:
        with (
            tc.tile_pool(name="const", bufs=1) as constp,
            tc.tile_pool(name="atp", bufs=2) as atp,
            tc.tile_pool(name="htp", bufs=2) as htp,
            tc.tile_pool(name="hnp", bufs=8) as hnp,
            tc.tile_pool(name="h4np", bufs=1) as h4np,
            tc.tile_pool(name="sortp", bufs=1) as sortp,
            tc.tile_pool(name="h4tp", bufs=1) as h4tp,
            tc.tile_pool(name="aggps", bufs=1, space="PSUM") as aggps,
            tc.tile_pool(name="wps", bufs=2, space="PSUM") as wps,
            tc.tile_pool(name="tps", bufs=2, space="PSUM") as tps,
        ):
            ft = constp.tile([128, NODES], BF16, tag="featT")
            nc.sync.dma_start(out=ft[:, 0:N], in_=featT[:, 0:N])
            wt = []
            for i in range(4):
                w = constp.tile([128, HID[i]], BF16, tag=f"w{i}")
                nc.sync.dma_start(out=w[:, :], in_=w_in[i][:, :])
                wt.append(w)

            at_tiles = [None] * GPC
            at0 = atp.tile([128, NCH, N], BF16, tag="at")
            for q in range(4):
                nc.sync.dma_start(
                    out=at0[:, q * 4 : (q + 1) * 4, :],
                    in_=at_in[0, :, q * 4 : (q + 1) * 4, :],
                )
            at_tiles[0] = at0

            bt = []
            for i in range(4):
                bb = constp.tile([HID[i], 1], F32, tag=f"b{i}")
                nc.gpsimd.dma_start(out=bb[:, :], in_=b_in[i][:, :])
                bt.append(bb)
            for g in range(1, GPC):
                nc.gpsimd.dma_start(
                    out=ft[:, g * N : (g + 1) * N], in_=featT[:, g * N : (g + 1) * N]
                )
            ident = constp.tile([128, 128], F32, tag="ident")
            nc.gpsimd.dma_start(out=ident[:, :], in_=ident_in[:, :])
            repm = constp.tile([16, 128], F32, tag="repm")
            nc.gpsimd.dma_start(out=repm[:, :], in_=rep_in[:, :])

            mvall = constp.tile([128, N], F32, tag="mvall")
            nc.vector.memset(mvall[:, :], -1e30)
            offsp = sortp.tile([128, 1], I32, tag="offsp")
            nc.gpsimd.iota(offsp[:, :], pattern=[[0, 1]], base=0, channel_multiplier=64)
            offspf = sortp.tile([128, 1], F32, tag="offspf")
            nc.vector.tensor_copy(offspf[:, :], offsp[:, :])
            idx16 = sortp.tile([128, 8], I16, tag="idx16")
            nc.vector.memset(idx16[:, :], -1)

            for g in range(GPC):
                if g > 0:
                    at = atp.tile([128, NCH, N], BF16, tag="at")
                    for q in range(4):
                        nc.sync.dma_start(
                            out=at[:, q * 4 : (q + 1) * 4, :],
                            in_=at_in[g, :, q * 4 : (q + 1) * 4, :],
                        )
                else:
                    at = at_tiles[0]

                hT_prev = None
                for li in range(4):
                    dout = HID[li]
                    hn_tiles = []
                    for cq in range(NCH // 4):
                        wp = wps.tile([128, 4, dout], F32, tag="wp")
                        for i in range(4):
                            c = cq * 4 + i
                            if li == 0:
                                lhsT = ft[:, g * N + c * 128 : g * N + (c + 1) * 128]
                            else:
                                lhsT = hT_prev[:, c * 128 : (c + 1) * 128]
                            nc.tensor.matmul(
                                wp[:, i, :], lhsT, wt[li][:, :dout],
                                start=True, stop=True,
                            )
                        hn = hnp.tile([128, 4, 128], BF16, tag="hn")
                        nc.vector.tensor_copy(hn[:, :, :dout], wp[:, :, :])
                        hn_tiles.append(hn)

                    agg = aggps.tile([128, N], F32, tag="agg")
                    for c in range(NCH):
                        lhsT = hn_tiles[c // 4][:, c % 4, :dout]
                        for dsp in range(N // 512):
                            nc.tensor.matmul(
                                agg[:dout, dsp * 512 : (dsp + 1) * 512],
                                lhsT,
                                at[:, c, dsp * 512 : (dsp + 1) * 512],
                                start=(c == 0),
                                stop=(c == NCH - 1),
                            )

                    if li < 3:
                        hT = htp.tile([128, N], BF16, tag="ht")
                        dst_t = hT
                    else:
                        dst_t = h4tp.tile([64, N], F32, tag="h4t")
                    for q in range(4):
                        sl = slice(q * 512, (q + 1) * 512)
                        nc.scalar.activation(
                            dst_t[:dout, sl], agg[:dout, sl], relu,
                            bias=bt[li][:, :],
                        )
                    if li < 3:
                        hT_prev = hT
                    else:
                        h4T = dst_t

                h4n = h4np.tile([128, NCH, 64], F32, tag="h4n")
                for c in range(NCH):
                    tp = tps.tile([128, 128], F32, tag="tp")
                    nc.tensor.transpose(
                        tp[:, :64], h4T[:, c * 128 : (c + 1) * 128], ident[:64, :64]
                    )
                    nc.vector.tensor_copy(h4n[:, c, :], tp[:, :64])
                mcg = sortp.tile([128, NCH], F32, tag="mcg")
                nc.vector.tensor_reduce(
                    mcg[:, :], h4n[:, :, :], axis=mybir.AxisListType.X, op=mxo
                )
                nc.sync.dma_start(
                    out=h4_hbm[g * N : (g + 1) * N, :].rearrange(
                        "(c p) f -> p c f", p=128
                    ),
                    in_=h4n[:, :, :],
                )
                tpg = tps.tile([128, 128], F32, tag="tp")
                nc.tensor.transpose(tpg[:NCH, :], mcg[:, :], ident[:, :])
                mtg = sortp.tile([NCH, 128], F32, tag="mtg")
                nc.vector.tensor_copy(mtg[:, :], tpg[:NCH, :])
                p0 = 32 * g
                nc.sync.dma_start(
                    out=mvall[p0 : p0 + 1, :].rearrange("o (c j) -> o c j", j=128),
                    in_=mtg[:, :],
                )

            mxs = sortp.tile([128, 8], F32, tag="mxs")
            mis = sortp.tile([128, 16], U32, tag="mis")
            nc.vector.max_with_indices(mxs[:, :], mis[:, 0:8], mvall[:, :])
            mv2 = sortp.tile([128, N], F32, tag="mv2")
            nc.vector.match_replace(mv2[:, :], mxs[:, :], mvall[:, :], -1e30)
            nc.vector.max_with_indices(mxs[:, :], mis[:, 8:16], mv2[:, :])

            idxf32 = sortp.tile([128, 16], F32, tag="idxf32")
            nc.vector.tensor_copy(idxf32[:, :], mis[:, :])
            idxo = sortp.tile([128, 16], F32, tag="idxo")
            nc.vector.tensor_scalar(
                idxo[:, :], idxf32[:, :], offspf[:, :], None, op0=mybir.AluOpType.add
            )
            tpi = tps.tile([128, 128], F32, tag="tp")
            nc.tensor.transpose(tpi[:16, :], idxo[:, :], ident[:, :])
            t1s = sortp.tile([16, 128], F32, tag="t1s")
            nc.vector.tensor_copy(t1s[:, :], tpi[:16, :])
            sel = t1s[:, :].rearrange("r (a b) -> r a b", b=32)[:, :, 0]
            tpr = tps.tile([128, 128], F32, tag="tp")
            nc.tensor.matmul(tpr[:, :GPC], repm[:, :], sel, start=True, stop=True)
            nc.vector.tensor_copy(idx16[:, 0:GPC], tpr[:, :GPC])

            gath = sortp.tile([128, 1, 64], F32, tag="gath")
            nc.gpsimd.dma_gather(
                gath[:, :, :],
                h4_hbm[:, :],
                idx16[:, :],
                num_idxs=128,
                num_idxs_reg=64,
                elem_size=64,
            )

            neg = sortp.tile([64, 64], F32, tag="neg")
            nc.vector.tensor_scalar(
                neg[:, :], gath[:64, 0, :], -1.0, None, op0=mybir.AluOpType.mult
            )
            desc = sortp.tile([64, 64], F32, tag="desc")
            pp0 = sortp.tile([64, 64], F32, tag="pp0")
            pp1 = sortp.tile([64, 64], F32, tag="pp1")
            pp = [pp0, pp1]
            cur = neg
            for r in range(8):
                nc.vector.max(desc[:, r * 8 : (r + 1) * 8], cur[:, :])
                if r < 7:
                    nxt = pp[r % 2]
                    nc.vector.match_replace(
                        nxt[:, :], desc[:, r * 8 : (r + 1) * 8], cur[:, :], -1e30
                    )
                    cur = nxt
            asc = sortp.tile([64, 64], F32, tag="asc")
            nc.vector.tensor_scalar(
                asc[:, :], desc[:, :], -1.0, None, op0=mybir.AluOpType.mult
            )

            nc.sync.dma_start(
                out=out_dram[:, :].rearrange("g (r f) -> (g r) f", f=64),
                in_=asc[:, :],
            )

    nc.compile()
    return nc


def _host_prep_fallback(inputs):
    feats = np.asarray(inputs["features"], np.float32)
    src = np.asarray(inputs["src"], np.int64)
    dst = np.asarray(inputs["dst"], np.int64)
    n_rand = B * N * DEG
    rs, rd = src[:n_rand], dst[:n_rand]

    ident = np.eye(128, dtype=np.float32)
    repmat = np.tile(np.eye(16, dtype=np.float32), (1, 8))
    in_maps = []
    for core in range(NCORES):
        at_core = np.empty((GPC, 128, NCH, N), dtype=ml_dtypes.bfloat16)
        for g in range(GPC):
            gb = core * GPC + g
            s = rs[gb * N * DEG : (gb + 1) * N * DEG] - gb * N
            d = rd[gb * N * DEG : (gb + 1) * N * DEG] - gb * N
            cnt = np.bincount(s * N + d, minlength=N * N).astype(np.float32)
            cnt = cnt.reshape(N, N)
            np.fill_diagonal(cnt, np.diagonal(cnt) + 1.0)
            odeg = cnt.sum(axis=1)
            ideg = cnt.sum(axis=0)
            od = (1.0 / np.sqrt(np.maximum(odeg, 1.0))).astype(np.float32)
            idg = (1.0 / np.sqrt(np.maximum(ideg, 1.0))).astype(np.float32)
            a = (od[:, None] * cnt) * idg[None, :]
            at_core[g] = (
                a.reshape(NCH, 128, N).transpose(1, 0, 2).astype(ml_dtypes.bfloat16)
            )
        fshard = np.ascontiguousarray(
            feats[core * NODES : (core + 1) * NODES].T
        ).astype(ml_dtypes.bfloat16)
        m = {"featT": fshard, "at": at_core, "ident": ident, "repmat": repmat}
        for i in range(4):
            m[f"w{i+1}"] = np.asarray(inputs[f"W{i+1}"], np.float32).astype(
                ml_dtypes.bfloat16
            )
            m[f"b{i+1}"] = np.asarray(inputs[f"b{i+1}"], np.float32).reshape(-1, 1)
        in_maps.append(m)
    return in_maps


def kernel(**inputs):
    zero_bias = all(
        not np.any(np.asarray(inputs[f"b{i+1}"], np.float32)) for i in range(4)
    )
    if zero_bias:
        in_maps, max_cnt = _host_prep_fast(inputs)
        if max_cnt <= 15.0:
            if "fast" not in _CACHE:
                _CACHE["fast"] = _build_graph_fast()
            nc = _CACHE["fast"]
        else:
            in_maps = None
    else:
        in_maps = None
    if in_maps is None:
        # arbitrary biases / huge multi-edge counts: exact dense bf16 path
        if "fb" not in _CACHE:
            _CACHE["fb"] = _build_graph_fallback()
        nc = _CACHE["fb"]
        in_maps = _host_prep_fallback(inputs)

    trace = bool(int(os.environ.get("KERNEL_TRACE", "0")))
    res = bass_utils.run_bass_kernel_spmd(
        nc, in_maps, core_ids=list(range(NCORES)), trace=trace
    )
    LAST["exec_time_ns"] = res.exec_time_ns
    out = np.concatenate([res.results[i]["out"] for i in range(NCORES)], axis=0)
    return out.astype(np.float32)
